# revision 20
# baseline (speedup 1.0000x reference)
"""MoE layer (dense-reference, top-2 routing) as a two-phase Bass kernel on 8 NeuronCores.

Phase 1 (data-parallel over tokens): router — logits = x @ gate_w.T in fp32,
softmax, top-2 indices + normalized combine weights, per-core softmax column
sums (for the aux loss `p` term).

Host glue: per-expert token gather (the data-dependent expert-parallel
sharding), aux-loss assembly from integer counts (f) and device p-partials.

Phase 2 (expert-parallel): each core computes its expert's SwiGLU
  y = (silu(x @ w1.T) * (x @ w3.T)) @ w2.T  scaled by the combine weight,
over its gathered tokens (bf16 matmuls, fp32 PSUM accumulation).

Host combine: scatter the two weighted expert rows back per token and add.
"""

import sys

if "/opt/trn_rl_repo" not in sys.path:
    sys.path.insert(0, "/opt/trn_rl_repo")

import numpy as np
import ml_dtypes

import concourse.bass as bass
import concourse.mybir as mybir
from concourse import bacc, tile
from concourse.bass import ts
from concourse.bass_utils import run_bass_kernel_spmd

BF16 = mybir.dt.bfloat16
F32 = mybir.dt.float32
U32 = mybir.dt.uint32

B, T, D, E, I = 4, 2048, 2048, 8, 1024
N = B * T            # 8192 tokens
TOP_K = 2
AUX_COEFF = 0.01
NCORES = 8
TSH = N // NCORES    # 1024 tokens per core in phase 1
CAP = 2176           # per-expert token capacity in phase 2 (max observed 2099)
P = 128


def _build_phase1() -> bass.Bass:
    """Router: per-core [D, TSH] x-shard (transposed) + [D, E] gate weights ->
    top-2 expert ids, normalized top-2 weights, softmax column sums."""
    nc = bacc.Bacc("TRN2", target_bir_lowering=False, name="moe_router")
    KD = D // P                      # 16 contraction tiles
    NT = TSH // P                    # 8 token tiles

    xt = nc.dram_tensor("xt", [D, TSH], F32, kind="ExternalInput")
    gwt = nc.dram_tensor("gwt", [D, E], F32, kind="ExternalInput")
    idx_out = nc.dram_tensor("idx", [TSH, TOP_K], U32, kind="ExternalOutput")
    w_out = nc.dram_tensor("w01", [TSH, TOP_K], F32, kind="ExternalOutput")
    p_out = nc.dram_tensor("p", [E, 1], F32, kind="ExternalOutput")

    with tile.TileContext(nc) as tc:
        with (
            tc.tile_pool(name="big", bufs=1) as big,
            tc.tile_pool(name="work", bufs=3) as work,
            tc.tile_pool(name="psum", bufs=3, space="PSUM") as psum,
            tc.tile_pool(name="psump", bufs=1, space="PSUM") as psump,
        ):
            xt_sb = big.tile([P, KD, TSH], F32)
            gwt_sb = big.tile([P, KD, E], F32)
            p_acc = big.tile([P, E], F32)
            ones_sb = big.tile([P, 1], F32)
            idx_stage = big.tile([P, NT, TOP_K], U32)
            w_stage = big.tile([P, NT, TOP_K], F32)

            xt_r = xt.rearrange("(ko p) t -> p ko t", p=P)
            nc.sync.dma_start(gwt_sb[:], gwt.rearrange("(ko p) e -> p ko e", p=P))
            # token-tile-major DMA (4 k-blocks per tile, ~256KB each): tile t's
            # matmuls + softmax chase the arrival of its own x block
            for t in range(NT):
                for kq in range(4):
                    nc.sync.dma_start(
                        xt_sb[:, kq * 4 : (kq + 1) * 4, ts(t, P)],
                        xt_r[:, kq * 4 : (kq + 1) * 4, ts(t, P)],
                    )
            nc.vector.memset(p_acc[:], 0.0)
            nc.vector.memset(ones_sb[:], 1.0)

            for t in range(NT):
                logit_ps = psum.tile([P, E], F32)
                for k in range(KD):
                    nc.tensor.matmul(
                        logit_ps[:],
                        xt_sb[:, k, ts(t, P)],
                        gwt_sb[:, k, :],
                        start=(k == 0),
                        stop=(k == KD - 1),
                    )
                # single consumer of the PSUM tile, everything else reads SBUF
                logit_sb = work.tile([P, E], F32)
                nc.vector.tensor_copy(out=logit_sb[:], in_=logit_ps[:])
                # softmax over the 8 experts (free dim)
                max8 = work.tile([P, 8], F32)
                nc.vector.max(out=max8[:], in_=logit_sb[:])
                neg_m = work.tile([P, 1], F32)
                nc.vector.tensor_scalar_mul(neg_m[:], max8[:, 0:1], -1.0)
                exp_sb = work.tile([P, E], F32)
                den = work.tile([P, 1], F32)
                nc.scalar.activation(
                    exp_sb[:],
                    logit_sb[:],
                    mybir.ActivationFunctionType.Exp,
                    bias=neg_m[:],
                    scale=1.0,
                    accum_out=den[:],
                )
                rden = work.tile([P, 1], F32)
                nc.vector.reciprocal(rden[:], den[:])
                scores = work.tile([P, E], F32)
                nc.vector.tensor_scalar_mul(scores[:], exp_sb[:], rden[:])
                # top-2 (sorted top-8 of the 8 scores)
                svals = work.tile([P, 8], F32)
                sidx = work.tile([P, 8], U32)
                nc.vector.max_with_indices(svals[:], sidx[:], scores[:])
                # normalized top-2 combine weights
                s01 = work.tile([P, 1], F32)
                nc.vector.tensor_add(s01[:], svals[:, 0:1], svals[:, 1:2])
                r01 = work.tile([P, 1], F32)
                nc.vector.reciprocal(r01[:], s01[:])
                nc.vector.tensor_scalar_mul(
                    w_stage[:, t, :], svals[:, 0:TOP_K], r01[:]
                )
                nc.vector.tensor_copy(out=idx_stage[:, t, :], in_=sidx[:, 0:TOP_K])
                # accumulate p partial sums (per-partition)
                nc.vector.tensor_add(p_acc[:], p_acc[:], scores[:])

            # batched export of indices and combine weights
            nc.sync.dma_start(
                idx_out.rearrange("(t p) c -> p t c", p=P), idx_stage[:]
            )
            nc.sync.dma_start(w_out.rearrange("(t p) c -> p t c", p=P), w_stage[:])

            # cross-partition reduce of p_acc via ones-matmul: [E,1]
            p_ps = psump.tile([E, 1], F32)
            nc.tensor.matmul(p_ps[:], p_acc[:], ones_sb[:], start=True, stop=True)
            p_sb = work.tile([E, 1], F32)
            nc.vector.tensor_copy(out=p_sb[:], in_=p_ps[:])
            nc.sync.dma_start(p_out[:], p_sb[:])

    nc.compile()
    return nc


def _build_phase2() -> bass.Bass:
    """Per-expert SwiGLU over CAP gathered tokens; output transposed [D, CAP],
    pre-scaled by the per-token combine weight."""
    nc = bacc.Bacc("TRN2", target_bir_lowering=False, name="moe_expert")
    KD = D // P       # 16 contraction tiles over D
    KI = I // P       # 8 contraction tiles over I
    CHUNK = 512
    chunks = []
    n0 = 0
    while n0 < CAP:
        nn = min(CHUNK, CAP - n0)
        chunks.append((n0, nn))
        n0 += nn

    xgt = nc.dram_tensor("xgt", [D, CAP], BF16, kind="ExternalInput")
    w1t = nc.dram_tensor("w1t", [D, I], BF16, kind="ExternalInput")
    w3t = nc.dram_tensor("w3t", [D, I], BF16, kind="ExternalInput")
    w2t = nc.dram_tensor("w2t", [I, D], BF16, kind="ExternalInput")
    wgb = nc.dram_tensor("wgb", [P, CAP], F32, kind="ExternalInput")
    ygt = nc.dram_tensor("ygt", [D, CAP], BF16, kind="ExternalOutput")

    xgt_r = xgt.rearrange("(ko p) t -> p ko t", p=P)
    ygt_r = ygt.rearrange("(ko p) t -> p ko t", p=P)

    with tile.TileContext(nc) as tc:
        with (
            tc.tile_pool(name="weights", bufs=1) as wpool,
            tc.tile_pool(name="xg", bufs=2) as xpool,
            tc.tile_pool(name="h", bufs=2) as hpool,
            tc.tile_pool(name="act", bufs=3) as apool,
            tc.tile_pool(name="out", bufs=3) as opool,
            tc.tile_pool(name="psh", bufs=2, space="PSUM") as psh,
            tc.tile_pool(name="psy", bufs=2, space="PSUM") as psy,
        ):
            w1_sb = wpool.tile([P, KD, I], BF16)
            w3_sb = wpool.tile([P, KD, I], BF16)
            w2_sb = wpool.tile([P, KI, D], BF16)
            wg_sb = wpool.tile([P, CAP], F32)

            w1t_r = w1t.rearrange("(ko p) i -> p ko i", p=P)
            w3t_r = w3t.rearrange("(ko p) i -> p ko i", p=P)
            w2t_r = w2t.rearrange("(ko p) d -> p ko d", p=P)

            def load_xg(ci):
                n0, nn = chunks[ci]
                xg_sb = xpool.tile([P, KD, CHUNK], BF16)
                # per-subtile DMAs spread the load across all HW queues
                for k in range(KD):
                    nc.sync.dma_start(
                        xg_sb[:, k, :nn], xgt_r[:, k, n0 : n0 + nn]
                    )
                return xg_sb

            # chunk 0's activations first (needed by the very first matmul),
            # then weights split per contraction subtile
            xg0_sb = load_xg(0)
            for k in range(KD):
                nc.sync.dma_start(w1_sb[:, k, :], w1t_r[:, k, :])
            for k in range(KD):
                nc.sync.dma_start(w3_sb[:, k, :], w3t_r[:, k, :])
            for k in range(KI):
                nc.sync.dma_start(w2_sb[:, k, :], w2t_r[:, k, :])
            nc.sync.dma_start(wg_sb[:], wgb[:])

            def emit_h(ci):
                n0, nn = chunks[ci]
                xg_sb = xg0_sb if ci == 0 else load_xg(ci)
                h_sb = hpool.tile([P, KI, CHUNK], BF16)
                for it in range(KI):
                    ps1 = psh.tile([P, CHUNK], F32)
                    ps3 = psh.tile([P, CHUNK], F32)
                    for k in range(KD):
                        nc.tensor.matmul(
                            ps1[:, :nn],
                            w1_sb[:, k, ts(it, P)],
                            xg_sb[:, k, :nn],
                            start=(k == 0),
                            stop=(k == KD - 1),
                        )
                    for k in range(KD):
                        nc.tensor.matmul(
                            ps3[:, :nn],
                            w3_sb[:, k, ts(it, P)],
                            xg_sb[:, k, :nn],
                            start=(k == 0),
                            stop=(k == KD - 1),
                        )
                    sg = apool.tile([P, CHUNK], BF16)
                    nc.scalar.activation(
                        sg[:, :nn], ps1[:, :nn], mybir.ActivationFunctionType.Sigmoid
                    )
                    sil = apool.tile([P, CHUNK], BF16)
                    nc.vector.tensor_mul(sil[:, :nn], sg[:, :nn], ps1[:, :nn])
                    nc.vector.tensor_mul(h_sb[:, it, :nn], sil[:, :nn], ps3[:, :nn])
                return h_sb

            def emit_y(ci, h_sb):
                n0, nn = chunks[ci]
                for dt_ in range(D // P):
                    ps = psy.tile([P, CHUNK], F32)
                    for it in range(KI):
                        nc.tensor.matmul(
                            ps[:, :nn],
                            w2_sb[:, it, ts(dt_, P)],
                            h_sb[:, it, :nn],
                            start=(it == 0),
                            stop=(it == KI - 1),
                        )
                    out_sb = opool.tile([P, CHUNK], BF16)
                    nc.vector.tensor_mul(
                        out_sb[:, :nn], ps[:, :nn], wg_sb[:, n0 : n0 + nn]
                    )
                    nc.sync.dma_start(ygt_r[:, dt_, n0 : n0 + nn], out_sb[:, :nn])

            # software pipeline: h(ci+1) is emitted before y(ci) so the PE
            # never waits on the silu/mul epilogue of the current chunk
            prev = None
            for ci in range(len(chunks)):
                h_sb = emit_h(ci)
                if prev is not None:
                    emit_y(ci - 1, prev)
                prev = h_sb
            emit_y(len(chunks) - 1, prev)

    nc.compile()
    return nc


_NC1 = None
_NC2 = None


def _get_programs():
    global _NC1, _NC2
    if _NC1 is None:
        _NC1 = _build_phase1()
    if _NC2 is None:
        _NC2 = _build_phase2()
    return _NC1, _NC2


def _silu_np(v):
    return v / (1.0 + np.exp(-v))


def kernel(x, gate_w, w1, w3, w2, _trace=False, _results=None):
    nc1, nc2 = _get_programs()
    xf = np.ascontiguousarray(np.asarray(x, dtype=np.float32).reshape(N, D))
    gate_w = np.asarray(gate_w, dtype=np.float32)
    w1 = np.asarray(w1, dtype=np.float32)
    w3 = np.asarray(w3, dtype=np.float32)
    w2 = np.asarray(w2, dtype=np.float32)

    # ---- phase 1: routing (data-parallel over tokens) ----
    xT = np.ascontiguousarray(xf.T)                      # [D, N]
    gwT = np.ascontiguousarray(gate_w.T)                 # [D, E]
    in_maps1 = [
        {"xt": np.ascontiguousarray(xT[:, c * TSH : (c + 1) * TSH]), "gwt": gwT}
        for c in range(NCORES)
    ]
    res1 = run_bass_kernel_spmd(nc1, in_maps1, core_ids=list(range(NCORES)),
                                trace=_trace)
    if _results is not None:
        _results.append(res1)
    idx = np.concatenate([r["idx"] for r in res1.results], axis=0).astype(np.int64)
    w01 = np.concatenate([r["w01"] for r in res1.results], axis=0)   # [N, 2] f32
    p_sum = np.sum([r["p"][:, 0] for r in res1.results], axis=0)     # [E]

    # ---- aux loss ----
    counts = np.bincount(idx.ravel(), minlength=E)
    f = counts.astype(np.float64) / N
    p = p_sum.astype(np.float64) / N
    aux_loss = np.float32(AUX_COEFF * E * np.sum(f * p))

    # ---- host dispatch: group (token, slot) pairs by expert ----
    flat_idx = idx.reshape(-1)                           # token-major, slot minor
    order = np.argsort(flat_idx, kind="stable")
    tok_of = order // TOP_K
    slot_of = order % TOP_K
    seg = np.searchsorted(flat_idx[order], np.arange(E + 1))

    xf_bf = xf.astype(ml_dtypes.bfloat16)
    in_maps2 = []
    overflow = []                                        # (expert, tok_ids, weights)
    seg_tok = []
    for e in range(E):
        rows = tok_of[seg[e] : seg[e + 1]]
        slots = slot_of[seg[e] : seg[e + 1]]
        if len(rows) > CAP:
            overflow.append((e, rows[CAP:], w01[rows[CAP:], slots[CAP:]]))
            rows, slots = rows[:CAP], slots[:CAP]
        seg_tok.append(rows)
        xg = np.zeros((CAP, D), dtype=ml_dtypes.bfloat16)
        xg[: len(rows)] = xf_bf[rows]
        wg = np.zeros((CAP,), dtype=np.float32)
        wg[: len(rows)] = w01[rows, slots]
        in_maps2.append(
            {
                "xgt": np.ascontiguousarray(xg.T),
                "w1t": np.ascontiguousarray(w1[e].T.astype(ml_dtypes.bfloat16)),
                "w3t": np.ascontiguousarray(w3[e].T.astype(ml_dtypes.bfloat16)),
                "w2t": np.ascontiguousarray(w2[e].T.astype(ml_dtypes.bfloat16)),
                "wgb": np.ascontiguousarray(
                    np.broadcast_to(wg[None, :], (P, CAP))
                ),
            }
        )

    # ---- phase 2: expert-parallel SwiGLU ----
    res2 = run_bass_kernel_spmd(nc2, in_maps2, core_ids=list(range(NCORES)),
                                trace=_trace)
    if _results is not None:
        _results.append(res2)

    # ---- combine: scatter the two weighted expert outputs back per token ----
    y_parts = [
        np.asarray(res2.results[e]["ygt"]).T[: len(seg_tok[e])].astype(np.float32)
        for e in range(E)
    ]
    Y = np.concatenate(y_parts, axis=0)                  # [~N*2, D], expert order
    glob_tok = tok_of.copy()
    glob_slot = slot_of.copy()
    # positions: y_parts rows follow `order` except truncated segments
    kept = np.ones(len(order), dtype=bool)
    for e, rows, _w in overflow:
        kept[seg[e] + CAP : seg[e + 1]] = False
    glob_tok = glob_tok[kept]
    glob_slot = glob_slot[kept]
    inv = np.full((N, TOP_K), -1, dtype=np.int64)
    inv[glob_tok, glob_slot] = np.arange(len(glob_tok))
    zrow = np.zeros((1, D), dtype=np.float32)
    Yz = np.concatenate([Y, zrow], axis=0)               # -1 maps to zero row
    out = Yz[inv[:, 0]] + Yz[inv[:, 1]]

    # overflow fallback (never expected with CAP=2176): dense numpy for the few rows
    for e, rows, wts in overflow:
        xe = xf[rows]
        h = _silu_np(xe @ w1[e].T) * (xe @ w3[e].T)
        out[rows] += (h @ w2[e].T) * wts[:, None]

    return out.reshape(B, T, D).astype(np.float32), aux_loss


# revision 24
# speedup vs baseline: 1.0010x; 1.0010x over previous
"""MoE layer (dense-reference, top-2 routing) as a two-phase Bass kernel on 8 NeuronCores.

Phase 1 (data-parallel over tokens): router — logits = x @ gate_w.T in fp32,
softmax, top-2 indices + normalized combine weights, per-core softmax column
sums (for the aux loss `p` term).

Host glue: per-expert token gather (the data-dependent expert-parallel
sharding), aux-loss assembly from integer counts (f) and device p-partials.

Phase 2 (expert-parallel): each core computes its expert's SwiGLU
  y = (silu(x @ w1.T) * (x @ w3.T)) @ w2.T  scaled by the combine weight,
over its gathered tokens (bf16 matmuls, fp32 PSUM accumulation).

Host combine: scatter the two weighted expert rows back per token and add.
"""

import sys

if "/opt/trn_rl_repo" not in sys.path:
    sys.path.insert(0, "/opt/trn_rl_repo")

import numpy as np
import ml_dtypes

import concourse.bass as bass
import concourse.mybir as mybir
from concourse import bacc, tile
from concourse.bass import ts
from concourse.bass_utils import run_bass_kernel_spmd

BF16 = mybir.dt.bfloat16
F32 = mybir.dt.float32
U32 = mybir.dt.uint32

B, T, D, E, I = 4, 2048, 2048, 8, 1024
N = B * T            # 8192 tokens
TOP_K = 2
AUX_COEFF = 0.01
NCORES = 8
TSH = N // NCORES    # 1024 tokens per core in phase 1
CAP = 2176           # per-expert token capacity in phase 2 (max observed 2099)
P = 128


def _build_phase1() -> bass.Bass:
    """Router: per-core [D, TSH] x-shard (transposed) + [D, E] gate weights ->
    top-2 expert ids, normalized top-2 weights, softmax column sums."""
    nc = bacc.Bacc("TRN2", target_bir_lowering=False, name="moe_router")
    KD = D // P                      # 16 contraction tiles
    NT = TSH // P                    # 8 token tiles

    xt = nc.dram_tensor("xt", [D, TSH], F32, kind="ExternalInput")
    gwt = nc.dram_tensor("gwt", [D, E], F32, kind="ExternalInput")
    idx_out = nc.dram_tensor("idx", [TSH, TOP_K], U32, kind="ExternalOutput")
    w_out = nc.dram_tensor("w01", [TSH, TOP_K], F32, kind="ExternalOutput")
    p_out = nc.dram_tensor("p", [E, 1], F32, kind="ExternalOutput")

    with tile.TileContext(nc) as tc:
        with (
            tc.tile_pool(name="big", bufs=1) as big,
            tc.tile_pool(name="work", bufs=3) as work,
            tc.tile_pool(name="psum", bufs=3, space="PSUM") as psum,
            tc.tile_pool(name="psump", bufs=1, space="PSUM") as psump,
        ):
            xt_sb = big.tile([P, KD, TSH], F32)
            gwt_sb = big.tile([P, KD, E], F32)
            p_acc = big.tile([P, E], F32)
            ones_sb = big.tile([P, 1], F32)
            idx_stage = big.tile([P, NT, TOP_K], U32)
            w_stage = big.tile([P, NT, TOP_K], F32)

            xt_r = xt.rearrange("(ko p) t -> p ko t", p=P)
            nc.sync.dma_start(gwt_sb[:], gwt.rearrange("(ko p) e -> p ko e", p=P))
            # token-tile-major DMA (4 k-blocks per tile, ~256KB each): tile t's
            # matmuls + softmax chase the arrival of its own x block
            for t in range(NT):
                for kq in range(4):
                    nc.sync.dma_start(
                        xt_sb[:, kq * 4 : (kq + 1) * 4, ts(t, P)],
                        xt_r[:, kq * 4 : (kq + 1) * 4, ts(t, P)],
                    )
            nc.vector.memset(p_acc[:], 0.0)
            nc.vector.memset(ones_sb[:], 1.0)

            for t in range(NT):
                logit_ps = psum.tile([P, E], F32)
                for k in range(KD):
                    nc.tensor.matmul(
                        logit_ps[:],
                        xt_sb[:, k, ts(t, P)],
                        gwt_sb[:, k, :],
                        start=(k == 0),
                        stop=(k == KD - 1),
                    )
                # logits are bounded (|l| < ~6 for these scales) so exp needs
                # no max-subtraction; the softmax denominator cancels in both
                # the top-k order and the top-2 weight ratio
                exp_sb = work.tile([P, E], F32)
                den = work.tile([P, 1], F32)
                nc.scalar.activation(
                    exp_sb[:],
                    logit_ps[:],
                    mybir.ActivationFunctionType.Exp,
                    accum_out=den[:],
                )
                # top-2 straight on the exponentials (same order as softmax)
                svals = work.tile([P, 8], F32)
                sidx = work.tile([P, 8], U32)
                nc.vector.max_with_indices(svals[:], sidx[:], exp_sb[:])
                s01 = work.tile([P, 1], F32)
                nc.vector.tensor_add(s01[:], svals[:, 0:1], svals[:, 1:2])
                r01 = work.tile([P, 1], F32)
                nc.vector.reciprocal(r01[:], s01[:])
                nc.vector.tensor_scalar_mul(
                    w_stage[:, t, :], svals[:, 0:TOP_K], r01[:]
                )
                nc.vector.tensor_copy(out=idx_stage[:, t, :], in_=sidx[:, 0:TOP_K])
                # true softmax scores only needed for the p partial sums
                rden = work.tile([P, 1], F32)
                nc.vector.reciprocal(rden[:], den[:])
                scores = work.tile([P, E], F32)
                nc.vector.tensor_scalar_mul(scores[:], exp_sb[:], rden[:])
                nc.vector.tensor_add(p_acc[:], p_acc[:], scores[:])

            # batched export of indices and combine weights
            nc.sync.dma_start(
                idx_out.rearrange("(t p) c -> p t c", p=P), idx_stage[:]
            )
            nc.sync.dma_start(w_out.rearrange("(t p) c -> p t c", p=P), w_stage[:])

            # cross-partition reduce of p_acc via ones-matmul: [E,1]
            p_ps = psump.tile([E, 1], F32)
            nc.tensor.matmul(p_ps[:], p_acc[:], ones_sb[:], start=True, stop=True)
            p_sb = work.tile([E, 1], F32)
            nc.vector.tensor_copy(out=p_sb[:], in_=p_ps[:])
            nc.sync.dma_start(p_out[:], p_sb[:])

    nc.compile()
    return nc


def _build_phase2() -> bass.Bass:
    """Per-expert SwiGLU over CAP gathered tokens; output transposed [D, CAP],
    pre-scaled by the per-token combine weight."""
    nc = bacc.Bacc("TRN2", target_bir_lowering=False, name="moe_expert")
    KD = D // P       # 16 contraction tiles over D
    KI = I // P       # 8 contraction tiles over I
    # balanced chunk sizes: sim cost is chunking-invariant (total rows fixed)
    # but on HW a thin tail chunk pays the per-matmul LDW/dispatch floor, so
    # keep every chunk's free dim large (and <=512 for one PSUM bank)
    CHUNK = 512
    sizes = [448, 448, 448, 448, 384]
    assert sum(sizes) == CAP
    chunks = []
    n0 = 0
    for nn in sizes:
        chunks.append((n0, nn))
        n0 += nn


    xgt = nc.dram_tensor("xgt", [D, CAP], BF16, kind="ExternalInput")
    w1t = nc.dram_tensor("w1t", [D, I], BF16, kind="ExternalInput")
    w3t = nc.dram_tensor("w3t", [D, I], BF16, kind="ExternalInput")
    w2t = nc.dram_tensor("w2t", [I, D], BF16, kind="ExternalInput")
    wgb = nc.dram_tensor("wgb", [P, CAP], F32, kind="ExternalInput")
    ygt = nc.dram_tensor("ygt", [D, CAP], BF16, kind="ExternalOutput")

    xgt_r = xgt.rearrange("(ko p) t -> p ko t", p=P)
    ygt_r = ygt.rearrange("(ko p) t -> p ko t", p=P)

    with tile.TileContext(nc) as tc:
        with (
            tc.tile_pool(name="weights", bufs=1) as wpool,
            tc.tile_pool(name="xg", bufs=2) as xpool,
            tc.tile_pool(name="h", bufs=2) as hpool,
            tc.tile_pool(name="act", bufs=3) as apool,
            tc.tile_pool(name="out", bufs=3) as opool,
            tc.tile_pool(name="psh", bufs=2, space="PSUM") as psh,
            tc.tile_pool(name="psy", bufs=2, space="PSUM") as psy,
        ):
            w1_sb = wpool.tile([P, KD, I], BF16)
            w3_sb = wpool.tile([P, KD, I], BF16)
            w2_sb = wpool.tile([P, KI, D], BF16)
            wg_sb = wpool.tile([P, CAP], F32)

            w1t_r = w1t.rearrange("(ko p) i -> p ko i", p=P)
            w3t_r = w3t.rearrange("(ko p) i -> p ko i", p=P)
            w2t_r = w2t.rearrange("(ko p) d -> p ko d", p=P)

            def load_xg(ci):
                n0, nn = chunks[ci]
                xg_sb = xpool.tile([P, KD, CHUNK], BF16)
                # per-subtile DMAs spread the load across all HW queues
                for k in range(KD):
                    nc.sync.dma_start(
                        xg_sb[:, k, :nn], xgt_r[:, k, n0 : n0 + nn]
                    )
                return xg_sb

            # chunk 0's activations first (needed by the very first matmul),
            # then weights split per contraction subtile
            xg0_sb = load_xg(0)
            for k in range(KD):
                nc.sync.dma_start(w1_sb[:, k, :], w1t_r[:, k, :])
            for k in range(KD):
                nc.sync.dma_start(w3_sb[:, k, :], w3t_r[:, k, :])
            for k in range(KI):
                nc.sync.dma_start(w2_sb[:, k, :], w2t_r[:, k, :])
            nc.sync.dma_start(wg_sb[:], wgb[:])

            def emit_h(ci):
                n0, nn = chunks[ci]
                xg_sb = xg0_sb if ci == 0 else load_xg(ci)
                h_sb = hpool.tile([P, KI, CHUNK], BF16)
                for it in range(KI):
                    ps1 = psh.tile([P, CHUNK], F32)
                    ps3 = psh.tile([P, CHUNK], F32)
                    for k in range(KD):
                        nc.tensor.matmul(
                            ps1[:, :nn],
                            w1_sb[:, k, ts(it, P)],
                            xg_sb[:, k, :nn],
                            start=(k == 0),
                            stop=(k == KD - 1),
                        )
                    for k in range(KD):
                        nc.tensor.matmul(
                            ps3[:, :nn],
                            w3_sb[:, k, ts(it, P)],
                            xg_sb[:, k, :nn],
                            start=(k == 0),
                            stop=(k == KD - 1),
                        )
                    sg = apool.tile([P, CHUNK], BF16)
                    nc.scalar.activation(
                        sg[:, :nn], ps1[:, :nn], mybir.ActivationFunctionType.Sigmoid
                    )
                    sil = apool.tile([P, CHUNK], BF16)
                    nc.vector.tensor_mul(sil[:, :nn], sg[:, :nn], ps1[:, :nn])
                    nc.vector.tensor_mul(h_sb[:, it, :nn], sil[:, :nn], ps3[:, :nn])
                return h_sb

            def emit_y(ci, h_sb):
                n0, nn = chunks[ci]
                for dt_ in range(D // P):
                    ps = psy.tile([P, CHUNK], F32)
                    for it in range(KI):
                        nc.tensor.matmul(
                            ps[:, :nn],
                            w2_sb[:, it, ts(dt_, P)],
                            h_sb[:, it, :nn],
                            start=(it == 0),
                            stop=(it == KI - 1),
                        )
                    out_sb = opool.tile([P, CHUNK], BF16)
                    nc.vector.tensor_mul(
                        out_sb[:, :nn], ps[:, :nn], wg_sb[:, n0 : n0 + nn]
                    )
                    nc.sync.dma_start(ygt_r[:, dt_, n0 : n0 + nn], out_sb[:, :nn])

            # software pipeline: h(ci+1) is emitted before y(ci) so the PE
            # never waits on the silu/mul epilogue of the current chunk
            prev = None
            for ci in range(len(chunks)):
                h_sb = emit_h(ci)
                if prev is not None:
                    emit_y(ci - 1, prev)
                prev = h_sb
            emit_y(len(chunks) - 1, prev)

    nc.compile()
    return nc


_NC1 = None
_NC2 = None


def _get_programs():
    global _NC1, _NC2
    if _NC1 is None:
        _NC1 = _build_phase1()
    if _NC2 is None:
        _NC2 = _build_phase2()
    return _NC1, _NC2


def _silu_np(v):
    return v / (1.0 + np.exp(-v))


def kernel(x, gate_w, w1, w3, w2, _trace=False, _results=None):
    nc1, nc2 = _get_programs()
    xf = np.ascontiguousarray(np.asarray(x, dtype=np.float32).reshape(N, D))
    gate_w = np.asarray(gate_w, dtype=np.float32)
    w1 = np.asarray(w1, dtype=np.float32)
    w3 = np.asarray(w3, dtype=np.float32)
    w2 = np.asarray(w2, dtype=np.float32)

    # ---- phase 1: routing (data-parallel over tokens) ----
    xT = np.ascontiguousarray(xf.T)                      # [D, N]
    gwT = np.ascontiguousarray(gate_w.T)                 # [D, E]
    in_maps1 = [
        {"xt": np.ascontiguousarray(xT[:, c * TSH : (c + 1) * TSH]), "gwt": gwT}
        for c in range(NCORES)
    ]
    res1 = run_bass_kernel_spmd(nc1, in_maps1, core_ids=list(range(NCORES)),
                                trace=_trace)
    if _results is not None:
        _results.append(res1)
    idx = np.concatenate([r["idx"] for r in res1.results], axis=0).astype(np.int64)
    w01 = np.concatenate([r["w01"] for r in res1.results], axis=0)   # [N, 2] f32
    p_sum = np.sum([r["p"][:, 0] for r in res1.results], axis=0)     # [E]

    # ---- aux loss ----
    counts = np.bincount(idx.ravel(), minlength=E)
    f = counts.astype(np.float64) / N
    p = p_sum.astype(np.float64) / N
    aux_loss = np.float32(AUX_COEFF * E * np.sum(f * p))

    # ---- host dispatch: group (token, slot) pairs by expert ----
    flat_idx = idx.reshape(-1)                           # token-major, slot minor
    order = np.argsort(flat_idx, kind="stable")
    tok_of = order // TOP_K
    slot_of = order % TOP_K
    seg = np.searchsorted(flat_idx[order], np.arange(E + 1))

    xf_bf = xf.astype(ml_dtypes.bfloat16)
    in_maps2 = []
    overflow = []                                        # (expert, tok_ids, weights)
    seg_tok = []
    for e in range(E):
        rows = tok_of[seg[e] : seg[e + 1]]
        slots = slot_of[seg[e] : seg[e + 1]]
        if len(rows) > CAP:
            overflow.append((e, rows[CAP:], w01[rows[CAP:], slots[CAP:]]))
            rows, slots = rows[:CAP], slots[:CAP]
        seg_tok.append(rows)
        xg = np.zeros((CAP, D), dtype=ml_dtypes.bfloat16)
        xg[: len(rows)] = xf_bf[rows]
        wg = np.zeros((CAP,), dtype=np.float32)
        wg[: len(rows)] = w01[rows, slots]
        in_maps2.append(
            {
                "xgt": np.ascontiguousarray(xg.T),
                "w1t": np.ascontiguousarray(w1[e].T.astype(ml_dtypes.bfloat16)),
                "w3t": np.ascontiguousarray(w3[e].T.astype(ml_dtypes.bfloat16)),
                "w2t": np.ascontiguousarray(w2[e].T.astype(ml_dtypes.bfloat16)),
                "wgb": np.ascontiguousarray(
                    np.broadcast_to(wg[None, :], (P, CAP))
                ),
            }
        )

    # ---- phase 2: expert-parallel SwiGLU ----
    res2 = run_bass_kernel_spmd(nc2, in_maps2, core_ids=list(range(NCORES)),
                                trace=_trace)
    if _results is not None:
        _results.append(res2)

    # ---- combine: scatter the two weighted expert outputs back per token ----
    y_parts = [
        np.asarray(res2.results[e]["ygt"]).T[: len(seg_tok[e])].astype(np.float32)
        for e in range(E)
    ]
    Y = np.concatenate(y_parts, axis=0)                  # [~N*2, D], expert order
    glob_tok = tok_of.copy()
    glob_slot = slot_of.copy()
    # positions: y_parts rows follow `order` except truncated segments
    kept = np.ones(len(order), dtype=bool)
    for e, rows, _w in overflow:
        kept[seg[e] + CAP : seg[e + 1]] = False
    glob_tok = glob_tok[kept]
    glob_slot = glob_slot[kept]
    inv = np.full((N, TOP_K), -1, dtype=np.int64)
    inv[glob_tok, glob_slot] = np.arange(len(glob_tok))
    zrow = np.zeros((1, D), dtype=np.float32)
    Yz = np.concatenate([Y, zrow], axis=0)               # -1 maps to zero row
    out = Yz[inv[:, 0]] + Yz[inv[:, 1]]

    # overflow fallback (never expected with CAP=2176): dense numpy for the few rows
    for e, rows, wts in overflow:
        xe = xf[rows]
        h = _silu_np(xe @ w1[e].T) * (xe @ w3[e].T)
        out[rows] += (h @ w2[e].T) * wts[:, None]

    return out.reshape(B, T, D).astype(np.float32), aux_loss


# revision 35
# speedup vs baseline: 1.0983x; 1.0972x over previous
"""MoE layer (dense-reference, top-2 routing) as a two-phase Bass kernel on 8 NeuronCores.

Phase 1 (data-parallel over tokens): router — logits = x @ gate_w.T in fp32,
softmax, top-2 indices + normalized combine weights, per-core softmax column
sums (for the aux loss `p` term).

Host glue: per-expert token gather (the data-dependent expert-parallel
sharding), aux-loss assembly from integer counts (f) and device p-partials.

Phase 2 (expert-parallel): each core computes its expert's SwiGLU
  y = (silu(x @ w1.T) * (x @ w3.T)) @ w2.T  scaled by the combine weight,
over its gathered tokens (bf16 matmuls, fp32 PSUM accumulation).

Host combine: scatter the two weighted expert rows back per token and add.
"""

import sys

if "/opt/trn_rl_repo" not in sys.path:
    sys.path.insert(0, "/opt/trn_rl_repo")

import numpy as np
import ml_dtypes

import concourse.bass as bass
import concourse.mybir as mybir
from concourse import bacc, tile
from concourse.bass import ts
from concourse.bass_utils import run_bass_kernel_spmd

BF16 = mybir.dt.bfloat16
F32 = mybir.dt.float32
U32 = mybir.dt.uint32

B, T, D, E, I = 4, 2048, 2048, 8, 1024
N = B * T            # 8192 tokens
TOP_K = 2
AUX_COEFF = 0.01
NCORES = 8
TSH = N // NCORES    # 1024 tokens per core in phase 1
CAP = 2112           # per-expert token capacity in phase 2 (max observed 2099;
                     # host numpy fallback covers any overflow)
P = 128


def _build_phase1() -> bass.Bass:
    """Router: per-core [D, TSH] x-shard (transposed) + [D, E] gate weights ->
    top-2 expert ids, normalized top-2 weights, softmax column sums."""
    nc = bacc.Bacc("TRN2", target_bir_lowering=False, name="moe_router")
    KD = D // P                      # 16 contraction tiles
    NT = TSH // P                    # 8 token tiles

    xt = nc.dram_tensor("xt", [D, TSH], F32, kind="ExternalInput")
    gwt = nc.dram_tensor("gwt", [D, E], F32, kind="ExternalInput")
    idx_out = nc.dram_tensor("idx", [TSH, TOP_K], U32, kind="ExternalOutput")
    w_out = nc.dram_tensor("w01", [TSH, TOP_K], F32, kind="ExternalOutput")
    p_out = nc.dram_tensor("p", [E, 1], F32, kind="ExternalOutput")

    with tile.TileContext(nc) as tc:
        with (
            tc.tile_pool(name="big", bufs=1) as big,
            tc.tile_pool(name="work", bufs=6) as work,
            tc.tile_pool(name="psum", bufs=4, space="PSUM") as psum,
            tc.tile_pool(name="psump", bufs=1, space="PSUM") as psump,
        ):
            xt_sb = big.tile([P, KD, TSH], F32)
            gwt_sb = big.tile([P, KD, E], F32)
            p_acc = big.tile([P, E], F32)
            ones_sb = big.tile([P, 1], F32)
            idx_stage = big.tile([P, NT, TOP_K], U32)
            w_stage = big.tile([P, NT, TOP_K], F32)

            xt_r = xt.rearrange("(ko p) t -> p ko t", p=P)
            nc.sync.dma_start(gwt_sb[:], gwt.rearrange("(ko p) e -> p ko e", p=P))
            # token-tile-major DMA (4 k-blocks per tile, ~256KB each): tile t's
            # matmuls + softmax chase the arrival of its own x block. Issue
            # from two engines in parallel — per-dma_start issue cost (~0.8us)
            # on a single engine would otherwise pace the whole kernel.
            for t in range(NT):
                eng = (nc.sync, nc.scalar, nc.gpsimd)[t % 3]
                for kq in range(4):
                    eng.dma_start(
                        xt_sb[:, kq * 4 : (kq + 1) * 4, ts(t, P)],
                        xt_r[:, kq * 4 : (kq + 1) * 4, ts(t, P)],
                    )
            nc.vector.memset(p_acc[:], 0.0)
            nc.vector.memset(ones_sb[:], 1.0)

            for t in range(NT):
                logit_ps = psum.tile([P, E], F32)
                for k in range(KD):
                    nc.tensor.matmul(
                        logit_ps[:],
                        xt_sb[:, k, ts(t, P)],
                        gwt_sb[:, k, :],
                        start=(k == 0),
                        stop=(k == KD - 1),
                    )
                # logits are bounded (|l| < ~6 for these scales) so exp needs
                # no max-subtraction; the softmax denominator cancels in both
                # the top-k order and the top-2 weight ratio
                exp_sb = work.tile([P, E], F32)
                den = work.tile([P, 1], F32)
                nc.scalar.activation(
                    exp_sb[:],
                    logit_ps[:],
                    mybir.ActivationFunctionType.Exp,
                    accum_out=den[:],
                )
                # top-2 straight on the exponentials (same order as softmax)
                svals = work.tile([P, 8], F32)
                sidx = work.tile([P, 8], U32)
                nc.vector.max_with_indices(svals[:], sidx[:], exp_sb[:])
                s01 = work.tile([P, 1], F32)
                nc.vector.tensor_add(s01[:], svals[:, 0:1], svals[:, 1:2])
                r01 = work.tile([P, 1], F32)
                nc.vector.reciprocal(r01[:], s01[:])
                nc.vector.tensor_scalar_mul(
                    w_stage[:, t, :], svals[:, 0:TOP_K], r01[:]
                )
                nc.vector.tensor_copy(out=idx_stage[:, t, :], in_=sidx[:, 0:TOP_K])
                # true softmax scores only needed for the p partial sums
                rden = work.tile([P, 1], F32)
                nc.vector.reciprocal(rden[:], den[:])
                scores = work.tile([P, E], F32)
                nc.vector.tensor_scalar_mul(scores[:], exp_sb[:], rden[:])
                nc.vector.tensor_add(p_acc[:], p_acc[:], scores[:])

            # batched export of indices and combine weights off the SP engine
            nc.scalar.dma_start(
                idx_out.rearrange("(t p) c -> p t c", p=P), idx_stage[:]
            )
            nc.scalar.dma_start(w_out.rearrange("(t p) c -> p t c", p=P), w_stage[:])

            # cross-partition reduce of p_acc via ones-matmul: [E,1]
            p_ps = psump.tile([E, 1], F32)
            nc.tensor.matmul(p_ps[:], p_acc[:], ones_sb[:], start=True, stop=True)
            p_sb = work.tile([E, 1], F32)
            nc.vector.tensor_copy(out=p_sb[:], in_=p_ps[:])
            nc.scalar.dma_start(p_out[:], p_sb[:])

    nc.compile()
    return nc


def _build_phase2() -> bass.Bass:
    """Per-expert SwiGLU over CAP gathered tokens; output transposed [D, CAP],
    pre-scaled by the per-token combine weight."""
    nc = bacc.Bacc("TRN2", target_bir_lowering=False, name="moe_expert")
    KD = D // P       # 16 contraction tiles over D
    KI = I // P       # 8 contraction tiles over I
    # balanced chunk sizes: sim cost is chunking-invariant (total rows fixed)
    # but on HW a thin tail chunk pays the per-matmul LDW/dispatch floor, so
    # keep every chunk's free dim large (and <=512 for one PSUM bank)
    CHUNK = 512
    sizes = [448, 448, 448, 384, 384]
    assert sum(sizes) == CAP
    chunks = []
    n0 = 0
    for nn in sizes:
        chunks.append((n0, nn))
        n0 += nn


    xgt = nc.dram_tensor("xgt", [D, CAP], BF16, kind="ExternalInput")
    w1t = nc.dram_tensor("w1t", [D, I], BF16, kind="ExternalInput")
    w3t = nc.dram_tensor("w3t", [D, I], BF16, kind="ExternalInput")
    w2t = nc.dram_tensor("w2t", [I, D], BF16, kind="ExternalInput")
    wgb = nc.dram_tensor("wgb", [P, CAP], F32, kind="ExternalInput")
    ygt = nc.dram_tensor("ygt", [D, CAP], BF16, kind="ExternalOutput")

    xgt_r = xgt.rearrange("(ko p) t -> p ko t", p=P)
    ygt_r = ygt.rearrange("(ko p) t -> p ko t", p=P)

    with tile.TileContext(nc) as tc:
        with (
            tc.tile_pool(name="weights", bufs=1) as wpool,
            tc.tile_pool(name="xg", bufs=2) as xpool,
            tc.tile_pool(name="h", bufs=2) as hpool,
            tc.tile_pool(name="act", bufs=3) as apool,
            tc.tile_pool(name="out", bufs=3) as opool,
            tc.tile_pool(name="psh", bufs=2, space="PSUM") as psh,
            tc.tile_pool(name="psy", bufs=2, space="PSUM") as psy,
        ):
            w1_sb = wpool.tile([P, KD, I], BF16)
            w3_sb = wpool.tile([P, KD, I], BF16)
            w2_sb = wpool.tile([P, KI, D], BF16)
            wg_sb = wpool.tile([P, CAP], F32)

            w1t_r = w1t.rearrange("(ko p) i -> p ko i", p=P)
            w3t_r = w3t.rearrange("(ko p) i -> p ko i", p=P)
            w2t_r = w2t.rearrange("(ko p) d -> p ko d", p=P)

            def load_xg(ci):
                n0, nn = chunks[ci]
                xg_sb = xpool.tile([P, KD, CHUNK], BF16)
                # per-subtile DMAs spread the load across all HW queues;
                # issued from the scalar engine so they run in parallel with
                # the weight-stream issues on SP
                for k in range(KD):
                    nc.scalar.dma_start(
                        xg_sb[:, k, :nn], xgt_r[:, k, n0 : n0 + nn]
                    )
                return xg_sb

            # chunk 0's activations first (needed by the very first matmul),
            # then weights split per contraction subtile
            xg0_sb = load_xg(0)
            # w1 gates the first accumulation pass — dual-issue it (SP + POOL)
            for k in range(KD):
                eng = nc.sync if k % 2 == 0 else nc.gpsimd
                eng.dma_start(w1_sb[:, k, :], w1t_r[:, k, :])
            for k in range(KD):
                nc.sync.dma_start(w3_sb[:, k, :], w3t_r[:, k, :])
            for k in range(KI):
                nc.sync.dma_start(w2_sb[:, k, :], w2t_r[:, k, :])
            nc.sync.dma_start(wg_sb[:], wgb[:])

            def emit_h(ci):
                n0, nn = chunks[ci]
                xg_sb = xg0_sb if ci == 0 else load_xg(ci)
                h_sb = hpool.tile([P, KI, CHUNK], BF16)
                for it in range(KI):
                    ps1 = psh.tile([P, CHUNK], F32)
                    ps3 = psh.tile([P, CHUNK], F32)
                    for k in range(KD):
                        nc.tensor.matmul(
                            ps1[:, :nn],
                            w1_sb[:, k, ts(it, P)],
                            xg_sb[:, k, :nn],
                            start=(k == 0),
                            stop=(k == KD - 1),
                        )
                    for k in range(KD):
                        nc.tensor.matmul(
                            ps3[:, :nn],
                            w3_sb[:, k, ts(it, P)],
                            xg_sb[:, k, :nn],
                            start=(k == 0),
                            stop=(k == KD - 1),
                        )
                    sg = apool.tile([P, CHUNK], BF16)
                    nc.scalar.activation(
                        sg[:, :nn], ps1[:, :nn], mybir.ActivationFunctionType.Sigmoid
                    )
                    sil = apool.tile([P, CHUNK], BF16)
                    nc.vector.tensor_mul(sil[:, :nn], sg[:, :nn], ps1[:, :nn])
                    nc.vector.tensor_mul(h_sb[:, it, :nn], sil[:, :nn], ps3[:, :nn])
                return h_sb

            def emit_y(ci, h_sb):
                n0, nn = chunks[ci]
                for dt_ in range(D // P):
                    ps = psy.tile([P, CHUNK], F32)
                    for it in range(KI):
                        nc.tensor.matmul(
                            ps[:, :nn],
                            w2_sb[:, it, ts(dt_, P)],
                            h_sb[:, it, :nn],
                            start=(it == 0),
                            stop=(it == KI - 1),
                        )
                    out_sb = opool.tile([P, CHUNK], BF16)
                    nc.vector.tensor_mul(
                        out_sb[:, :nn], ps[:, :nn], wg_sb[:, n0 : n0 + nn]
                    )
                    nc.sync.dma_start(ygt_r[:, dt_, n0 : n0 + nn], out_sb[:, :nn])

            # software pipeline: h(ci+1) is emitted before y(ci) so the PE
            # never waits on the silu/mul epilogue of the current chunk
            prev = None
            for ci in range(len(chunks)):
                h_sb = emit_h(ci)
                if prev is not None:
                    emit_y(ci - 1, prev)
                prev = h_sb
            emit_y(len(chunks) - 1, prev)

    nc.compile()
    return nc


_NC1 = None
_NC2 = None


def _get_programs():
    global _NC1, _NC2
    if _NC1 is None:
        _NC1 = _build_phase1()
    if _NC2 is None:
        _NC2 = _build_phase2()
    return _NC1, _NC2


def _silu_np(v):
    return v / (1.0 + np.exp(-v))


def kernel(x, gate_w, w1, w3, w2, _trace=False, _results=None):
    nc1, nc2 = _get_programs()
    xf = np.ascontiguousarray(np.asarray(x, dtype=np.float32).reshape(N, D))
    gate_w = np.asarray(gate_w, dtype=np.float32)
    w1 = np.asarray(w1, dtype=np.float32)
    w3 = np.asarray(w3, dtype=np.float32)
    w2 = np.asarray(w2, dtype=np.float32)

    # ---- phase 1: routing (data-parallel over tokens) ----
    xT = np.ascontiguousarray(xf.T)                      # [D, N]
    gwT = np.ascontiguousarray(gate_w.T)                 # [D, E]
    in_maps1 = [
        {"xt": np.ascontiguousarray(xT[:, c * TSH : (c + 1) * TSH]), "gwt": gwT}
        for c in range(NCORES)
    ]
    res1 = run_bass_kernel_spmd(nc1, in_maps1, core_ids=list(range(NCORES)),
                                trace=_trace)
    if _results is not None:
        _results.append(res1)
    idx = np.concatenate([r["idx"] for r in res1.results], axis=0).astype(np.int64)
    w01 = np.concatenate([r["w01"] for r in res1.results], axis=0)   # [N, 2] f32
    p_sum = np.sum([r["p"][:, 0] for r in res1.results], axis=0)     # [E]

    # ---- aux loss ----
    counts = np.bincount(idx.ravel(), minlength=E)
    f = counts.astype(np.float64) / N
    p = p_sum.astype(np.float64) / N
    aux_loss = np.float32(AUX_COEFF * E * np.sum(f * p))

    # ---- host dispatch: group (token, slot) pairs by expert ----
    flat_idx = idx.reshape(-1)                           # token-major, slot minor
    order = np.argsort(flat_idx, kind="stable")
    tok_of = order // TOP_K
    slot_of = order % TOP_K
    seg = np.searchsorted(flat_idx[order], np.arange(E + 1))

    xf_bf = xf.astype(ml_dtypes.bfloat16)
    in_maps2 = []
    overflow = []                                        # (expert, tok_ids, weights)
    seg_tok = []
    for e in range(E):
        rows = tok_of[seg[e] : seg[e + 1]]
        slots = slot_of[seg[e] : seg[e + 1]]
        if len(rows) > CAP:
            overflow.append((e, rows[CAP:], w01[rows[CAP:], slots[CAP:]]))
            rows, slots = rows[:CAP], slots[:CAP]
        seg_tok.append(rows)
        xg = np.zeros((CAP, D), dtype=ml_dtypes.bfloat16)
        xg[: len(rows)] = xf_bf[rows]
        wg = np.zeros((CAP,), dtype=np.float32)
        wg[: len(rows)] = w01[rows, slots]
        in_maps2.append(
            {
                "xgt": np.ascontiguousarray(xg.T),
                "w1t": np.ascontiguousarray(w1[e].T.astype(ml_dtypes.bfloat16)),
                "w3t": np.ascontiguousarray(w3[e].T.astype(ml_dtypes.bfloat16)),
                "w2t": np.ascontiguousarray(w2[e].T.astype(ml_dtypes.bfloat16)),
                "wgb": np.ascontiguousarray(
                    np.broadcast_to(wg[None, :], (P, CAP))
                ),
            }
        )

    # ---- phase 2: expert-parallel SwiGLU ----
    res2 = run_bass_kernel_spmd(nc2, in_maps2, core_ids=list(range(NCORES)),
                                trace=_trace)
    if _results is not None:
        _results.append(res2)

    # ---- combine: scatter the two weighted expert outputs back per token ----
    y_parts = [
        np.asarray(res2.results[e]["ygt"]).T[: len(seg_tok[e])].astype(np.float32)
        for e in range(E)
    ]
    Y = np.concatenate(y_parts, axis=0)                  # [~N*2, D], expert order
    glob_tok = tok_of.copy()
    glob_slot = slot_of.copy()
    # positions: y_parts rows follow `order` except truncated segments
    kept = np.ones(len(order), dtype=bool)
    for e, rows, _w in overflow:
        kept[seg[e] + CAP : seg[e + 1]] = False
    glob_tok = glob_tok[kept]
    glob_slot = glob_slot[kept]
    inv = np.full((N, TOP_K), -1, dtype=np.int64)
    inv[glob_tok, glob_slot] = np.arange(len(glob_tok))
    zrow = np.zeros((1, D), dtype=np.float32)
    Yz = np.concatenate([Y, zrow], axis=0)               # -1 maps to zero row
    out = Yz[inv[:, 0]] + Yz[inv[:, 1]]

    # overflow fallback (never expected with CAP=2176): dense numpy for the few rows
    for e, rows, wts in overflow:
        xe = xf[rows]
        h = _silu_np(xe @ w1[e].T) * (xe @ w3[e].T)
        out[rows] += (h @ w2[e].T) * wts[:, None]

    return out.reshape(B, T, D).astype(np.float32), aux_loss
